# revision 8
# baseline (speedup 1.0000x reference)
"""BitConvBlock kernel for 8x Trainium2 NeuronCores (SPMD, batch-sharded).

Reference computation (per sample):
  Wq = ternary-quantized W (BitNet b1.58: s = mean|W|, T = clip(round(W/(s+eps)),-1,1), Wq = s*T)
  y  = conv1d(x, Wq, pad=3)                      [B=16, Cout=512, L=8192]
  yn = GroupNorm(1 group, per-channel affine)(y)
  out= yn + sin(alpha*yn + phase)^2 / (alpha+eps)

Strategy:
  - Batch-parallel: 16 samples / 8 cores = 2 samples per core. GroupNorm is
    per-sample, so no collectives.
  - Conv as matmul: y[co,l] = sum_{k,ci} T[co,ci,k] * x[ci, l+k-3], scale s
    folded into the GN epsilon (GN normalization cancels a global scale;
    only eps needs rescaling by 1/s^2).
  - Weights are exactly representable in bf16 ({-1,0,1}); activations are
    split x = hi + lo (bf16 each) and accumulated with 2 matmuls per tile in
    fp32 PSUM -> ~1e-6 relative error at full bf16 PE throughput.
  - Conv output y is spilled to DRAM scratch tiles; after per-sample stats
    (sum / sum-of-squares accumulated on the fly), a second pass applies the
    GN affine + snake activation (ACT sin with magic-number range reduction).
"""
import os
import numpy as np
import ml_dtypes
from contextlib import ExitStack

# ---------------------------------------------------------------- constants
B, CIN, COUT, K, L = 16, 512, 512, 7, 8192
PAD = 3
EPS_Q, EPS_GN, EPS_A = 1e-5, 1e-5, 1e-9
NCORE = 8
BPC = B // NCORE          # samples per core
NCT = COUT // 128         # 4 co tiles
NCI = CIN // 128          # 4 ci tiles
LW = 512                  # conv l-window (one fp32 PSUM bank)
NLW = L // LW             # 16 windows
LP = L + 2 * PAD          # padded length 8198
BW = 1024                 # phase-B tile width
NBW = L // BW             # 8 phase-B tiles per (sample, co_t)
NELEM = COUT * L          # GN reduction size per sample
TWO_PI = 6.283185307179586
INV_2PI = 1.0 / TWO_PI
MAGIC = 12582912.0        # 1.5 * 2**23: float32 round-to-nearest-even trick

MM_DTYPE = os.environ.get("BITCONV_MM", "bf16")   # "bf16" (2-pass) | "fp32r"

_last_results = {}


def _ternary(W: np.ndarray):
    """Bit-exact replica of the reference's _ternary_quant (value part)."""
    s = None
    try:
        import jax
        import jax.numpy as jnp

        cpus = jax.devices("cpu")
        with jax.default_device(cpus[0]):
            s = float(jnp.mean(jnp.abs(jnp.asarray(W))))
    except Exception:
        s = float(np.mean(np.abs(W), dtype=np.float32))
    s32 = np.float32(s)
    q = (W / (s32 + np.float32(EPS_Q))).astype(np.float32)
    T = np.clip(np.rint(q), -1.0, 1.0).astype(np.float32)
    return T, float(s32)


def _build_and_run(in_maps, use_fp32r: bool, eps_eff: float, trace: bool):
    import concourse.bass as bass
    import concourse.tile as tile
    import concourse.mybir as mybir

    # walrus here accepts only one sync-wait per instruction; split waits.
    import bass_rust
    from concourse.vector_clock import ScopedClock, VectorClock

    _orig_commit = tile.TileContext._commit_and_lower
    _skip = (tile.BassTileRelease, tile.BassTileBranchHintPlaceholder,
             tile.BassTileCriticalSection)

    def _commit_split(self, inst, original_block, old_bb_map, bb_to_exit_bb):
        si = getattr(inst, "sync_info", None)
        if (si is not None and len(si.on_wait) > 1
                and not isinstance(inst, _skip)
                and not bass.is_branch_inst(inst)
                and inst.engine != mybir.EngineType.Unassigned):
            waits = list(si.on_wait)
            plain = [w for w in waits
                     if w.sync_type == "semaphore" and w.wait_reg is None]
            rest = [w for w in waits
                    if not (w.sync_type == "semaphore" and w.wait_reg is None)]
            if len(rest) <= 1 and plain:
                keep = rest if rest else [plain.pop()]
                for w in plain:
                    ev = mybir.InstEventSemaphore(
                        name=self.nc.get_next_instruction_name(), ins=[], outs=[])
                    ev.engine = inst.engine
                    ev.sync_info = bass_rust.SyncInfo(on_wait=[w], on_update=[])
                    self._commit_instruction(ev, lazy_reg_writes=False)
                inst.sync_info = bass_rust.SyncInfo(
                    on_wait=keep, on_update=list(si.on_update))
        return _orig_commit(self, inst, original_block, old_bb_map, bb_to_exit_bb)

    def _drain_split(self, tick_clock, wait_clock):
        g = tick_clock.global_clock
        n = len(g)
        for p in range(n):
            t = g[p]
            if t == 0:
                continue
            vec = [0] * n
            vec[p] = t
            d = self.nc.sync.drain()
            wait_clock.add_sem_waits(d.ins, ScopedClock({None: VectorClock(vec)}))
        self.nc.sync.drain()
        self.nc.all_engine_barrier()
        assert self.sems is not None
        popped = self.nc._tile_sem_poison_stack.pop()
        assert popped is self._sem_poison
        self.nc.clear_and_free_semaphores(list(self.sems.allocated().values()))
        self.nc.all_engine_barrier()

    tile.TileContext._commit_and_lower = _commit_split
    tile.TileContext._drain_and_barrier = _drain_split

    from concourse.bass_utils import run_bass_kernel_spmd

    f32 = mybir.dt.float32
    bf16 = mybir.dt.bfloat16
    f32r = mybir.dt.float32r
    AF = mybir.ActivationFunctionType
    ALU = mybir.AluOpType
    AX = mybir.AxisListType

    nc = bass.Bass("TRN2", target_bir_lowering=False, debug=False)

    if use_fp32r:
        x_in = nc.dram_tensor("xq", [BPC, NCI, 128, LP], f32r, kind="ExternalInput").ap()
        w_in = nc.dram_tensor("Wt", [128, K * NCI * NCT * 128], f32r, kind="ExternalInput").ap()
    else:
        xh_in = nc.dram_tensor("xh", [BPC, NCI, 128, LP], bf16, kind="ExternalInput").ap()
        xl_in = nc.dram_tensor("xl", [BPC, NCI, 128, LP], bf16, kind="ExternalInput").ap()
        w_in = nc.dram_tensor("Wt", [128, K * NCI * NCT * 128], bf16, kind="ExternalInput").ap()
    cc_in = nc.dram_tensor("cc", [128, 21], f32, kind="ExternalInput").ap()
    out_ap = nc.dram_tensor("out", [BPC, NCT, 128, L], f32, kind="ExternalOutput").ap()

    wdt = f32r if use_fp32r else bf16

    def widx(k, ci, ct):
        return ((ct * K + k) * NCI + ci) * 128

    with tile.TileContext(nc) as tc:
        with ExitStack() as ctx:
            wpool = ctx.enter_context(tc.tile_pool(name="w", bufs=1))
            cpool = ctx.enter_context(tc.tile_pool(name="consts", bufs=1))
            xpool = ctx.enter_context(tc.tile_pool(name="x", bufs=3))
            cps = ctx.enter_context(tc.tile_pool(name="cps", bufs=6, space="PSUM"))
            sps = ctx.enter_context(tc.tile_pool(name="sps", bufs=2, space="PSUM"))
            ypool = ctx.enter_context(tc.tile_pool(name="ysb", bufs=8))
            qpool = ctx.enter_context(tc.tile_pool(name="sqd", bufs=2))
            stpool = ctx.enter_context(tc.tile_pool(name="st", bufs=2))
            smpool = ctx.enter_context(tc.tile_pool(name="sm", bufs=2))
            bpool = ctx.enter_context(tc.tile_pool(name="bp", bufs=2 if use_fp32r else 3))
            ydram = ctx.enter_context(tc.tile_pool(name="ydram", bufs=2 * NCT * NBW, space="DRAM"))

            W_sb = wpool.tile([128, K * NCI * NCT * 128], wdt)
            wchunk = K * NCI * 128
            for ct in range(NCT):
                nc.sync.dma_start(W_sb[:, ct * wchunk:(ct + 1) * wchunk],
                                  w_in[:, ct * wchunk:(ct + 1) * wchunk])
            cc_sb = cpool.tile([128, 21], f32)
            nc.sync.dma_start(cc_sb[:], cc_in[:])
            gnw_c = cc_sb[:, 0:NCT]
            gnb_c = cc_sb[:, NCT:2 * NCT]
            alp_c = cc_sb[:, 2 * NCT:3 * NCT]
            phs_c = cc_sb[:, 3 * NCT:4 * NCT]
            sqi_c = cc_sb[:, 4 * NCT:5 * NCT]
            magic_c = cc_sb[:, 20:21]
            ones_sb = cpool.tile([128, 128], f32)
            nc.vector.memset(ones_sb[:], 1.0)

            for smp in range(BPC):
                st_sb = stpool.tile([128, 2 * NCT * NLW], f32, tag="st")
                ytiles = {}
                for ct in range(NCT):
                    for g in range(NBW):
                        ytiles[(ct, g)] = ydram.tile([128, BW], f32, name=f"yd_{smp}_{ct}_{g}", tag="yd")

                # ---- phase A: conv + stats ----
                for lw in range(NLW):
                    l0 = lw * LW
                    if use_fp32r:
                        xq_t = []
                        for ci in range(NCI):
                            t = xpool.tile([128, LW + 2 * PAD], f32r, tag=f"xq{ci}")
                            nc.sync.dma_start(t[:], x_in[smp, ci, :, l0:l0 + LW + 2 * PAD])
                            xq_t.append(t)
                    else:
                        xh_t, xl_t = [], []
                        for ci in range(NCI):
                            t = xpool.tile([128, LW + 2 * PAD], bf16, tag=f"xh{ci}")
                            nc.sync.dma_start(t[:], xh_in[smp, ci, :, l0:l0 + LW + 2 * PAD])
                            xh_t.append(t)
                            t = xpool.tile([128, LW + 2 * PAD], bf16, tag=f"xl{ci}")
                            nc.sync.dma_start(t[:], xl_in[smp, ci, :, l0:l0 + LW + 2 * PAD])
                            xl_t.append(t)

                    for ct in range(NCT):
                        ps = cps.tile([128, LW], f32, tag="cpsum")
                        for ci in range(NCI):
                            for k in range(K):
                                w_ap = W_sb[:, widx(k, ci, ct):widx(k, ci, ct) + 128]
                                first = ci == 0 and k == 0
                                last = ci == NCI - 1 and k == K - 1
                                if use_fp32r:
                                    nc.tensor.matmul(ps[:], w_ap, xq_t[ci][:, k:k + LW],
                                                     start=first, stop=last)
                                else:
                                    nc.tensor.matmul(ps[:], w_ap, xh_t[ci][:, k:k + LW],
                                                     start=first, stop=False)
                                    nc.tensor.matmul(ps[:], w_ap, xl_t[ci][:, k:k + LW],
                                                     start=False, stop=last)
                        idx = ct * NLW + lw
                        y_sb = ypool.tile([128, LW], f32, tag="ysb")
                        nc.vector.tensor_scalar(
                            y_sb[:], ps[:], 1.0, 0.0, ALU.mult, ALU.add,
                            accum_out=st_sb[:, idx:idx + 1])
                        sqd = qpool.tile([128, LW], f32, tag="sqd")
                        nc.scalar.activation(
                            sqd[:], ps[:], AF.Square,
                            accum_out=st_sb[:, NCT * NLW + idx:NCT * NLW + idx + 1])
                        g, o = lw // (BW // LW), (lw % (BW // LW)) * LW
                        nc.sync.dma_start(ytiles[(ct, g)][:, o:o + LW], y_sb[:])

                # ---- stats -> per-channel affine ----
                red = smpool.tile([128, 2], f32, tag="red")
                nc.vector.reduce_sum(red[:, 0:1], st_sb[:, 0:NCT * NLW], axis=AX.X)
                nc.vector.reduce_sum(red[:, 1:2], st_sb[:, NCT * NLW:2 * NCT * NLW], axis=AX.X)
                stps = sps.tile([128, 2], f32, tag="stps")
                nc.tensor.matmul(stps[:], ones_sb[:], red[:, 0:2], start=True, stop=True)
                mv = smpool.tile([128, 2], f32, tag="mv")
                nc.vector.tensor_scalar_mul(mv[:], stps[:], 1.0 / NELEM)
                musq = smpool.tile([128, 1], f32, tag="musq")
                nc.vector.tensor_mul(musq[:], mv[:, 0:1], mv[:, 0:1])
                var = smpool.tile([128, 1], f32, tag="var")
                nc.vector.tensor_sub(var[:], mv[:, 1:2], musq[:])
                nc.vector.tensor_scalar_add(var[:], var[:], float(eps_eff))
                std = smpool.tile([128, 1], f32, tag="std")
                nc.scalar.activation(std[:], var[:], AF.Sqrt)
                rv = smpool.tile([128, 1], f32, tag="rv")
                nc.vector.reciprocal(rv[:], std[:])
                Av = smpool.tile([128, NCT], f32, tag="Av")
                nc.vector.tensor_scalar_mul(Av[:], gnw_c, rv[:])
                negmu = smpool.tile([128, 1], f32, tag="negmu")
                nc.vector.tensor_scalar_mul(negmu[:], mv[:, 0:1], -1.0)
                Bv = smpool.tile([128, NCT], f32, tag="Bv")
                nc.vector.tensor_scalar_mul(Bv[:], Av[:], negmu[:])
                nc.vector.tensor_add(Bv[:], Bv[:], gnb_c)

                # ---- phase B: GN affine + snake ----
                for ct in range(NCT):
                    for g in range(NBW):
                        yin = bpool.tile([128, BW], f32, tag="yin")
                        nc.sync.dma_start(yin[:], ytiles[(ct, g)][:])
                        yn = bpool.tile([128, BW], f32, tag="yn")
                        nc.vector.tensor_scalar(
                            yn[:], yin[:], Av[:, ct:ct + 1], Bv[:, ct:ct + 1],
                            ALU.mult, ALU.add)
                        u = bpool.tile([128, BW], f32, tag="u")
                        nc.vector.tensor_scalar(
                            u[:], yn[:], alp_c[:, ct:ct + 1], phs_c[:, ct:ct + 1],
                            ALU.mult, ALU.add)
                        z = bpool.tile([128, BW], f32, tag="z")
                        nc.scalar.activation(z[:], u[:], AF.Identity,
                                             bias=magic_c, scale=INV_2PI)
                        t1 = bpool.tile([128, BW], f32, tag="t1")
                        nc.vector.tensor_scalar(
                            t1[:], z[:], -MAGIC, TWO_PI, ALU.add, ALU.mult)
                        redt = bpool.tile([128, BW], f32, tag="redt")
                        nc.gpsimd.tensor_sub(redt[:], u[:], t1[:])
                        sg = bpool.tile([128, BW], f32, tag="sg")
                        nc.scalar.activation(sg[:], redt[:], AF.Sin)
                        sq2 = bpool.tile([128, BW], f32, tag="sq2")
                        nc.scalar.activation(sq2[:], sg[:], AF.Square,
                                             scale=sqi_c[:, ct:ct + 1])
                        outt = bpool.tile([128, BW], f32, tag="outt")
                        nc.gpsimd.tensor_add(outt[:], yn[:], sq2[:])
                        nc.sync.dma_start(out_ap[smp, ct, :, g * BW:(g + 1) * BW], outt[:])

    if trace:
        _install_profile_shim()
    res = run_bass_kernel_spmd(nc, in_maps, list(range(NCORE)), trace=trace)
    return res


def _install_profile_shim():
    """Register antenv.axon_hooks so trace=True captures NTFF profiles via the
    axon .so (profiling only; never needed for plain execution)."""
    import sys, types, importlib.util

    if "antenv.axon_hooks" in sys.modules:
        return
    try:
        holder = {"hook": None}
        mod = types.ModuleType("antenv.axon_hooks")
        mod.set_axon_ntff_profile_hook = lambda h: holder.__setitem__("hook", h)
        mod.get_axon_ntff_profile_hook = lambda: holder["hook"]
        import antenv

        spec = importlib.util.spec_from_file_location(
            "trn_boot_shim", "/root/.axon_site/trn_agent_boot/trn_boot.py")
        boot = importlib.util.module_from_spec(spec)
        spec.loader.exec_module(boot)
        hook = boot._ntff_profile_via_ctypes("/opt/axon/libaxon_pjrt.so")
        if hook is None:
            return
        mod.set_axon_ntff_profile_hook(hook)
        sys.modules["antenv.axon_hooks"] = mod
        antenv.axon_hooks = mod
    except Exception:
        pass


def kernel(x, W, gn_w, gn_b, alpha, phase):
    x = np.asarray(x, dtype=np.float32)
    W = np.asarray(W, dtype=np.float32)
    gn_w = np.asarray(gn_w, dtype=np.float32)
    gn_b = np.asarray(gn_b, dtype=np.float32)
    alpha = np.asarray(alpha, dtype=np.float32)
    phase = np.asarray(phase, dtype=np.float32)

    use_fp32r = MM_DTYPE == "fp32r"
    trace = bool(int(os.environ.get("BITCONV_TRACE", "0")))

    T, s = _ternary(W)   # T in {-1,0,1}, conv scale s folded into GN eps
    eps_eff = float(EPS_GN / (np.float64(s) ** 2))

    # weight layout: Wt[ci_in_tile, (k, ci_t, co_t, co)] = T[co, ci, k]
    Tr = T.reshape(NCT, 128, NCI, 128, K)          # [co_t, co, ci_t, ci, k]
    # Wt[ci_in_tile, (co_t, k, ci_t, co)] = T[co, ci, k]   (ct-major for chunked DMA)
    Wt = np.ascontiguousarray(Tr.transpose(3, 0, 4, 2, 1)).reshape(128, -1)

    # padded activations, partition-tiled
    xp = np.zeros((B, CIN, LP), dtype=np.float32)
    xp[:, :, PAD:PAD + L] = x
    xp = xp.reshape(B, NCI, 128, LP)

    # per-channel constants [128, col]
    def tilec(v):
        return np.ascontiguousarray(v.reshape(NCT, 128).T)  # [128, NCT]

    sqinv = np.sqrt(1.0 / (alpha.astype(np.float64) + EPS_A)).astype(np.float32)
    cc = np.zeros((128, 21), dtype=np.float32)
    cc[:, 0:NCT] = tilec(gn_w)
    cc[:, NCT:2 * NCT] = tilec(gn_b)
    cc[:, 2 * NCT:3 * NCT] = tilec(alpha)
    cc[:, 3 * NCT:4 * NCT] = tilec(phase)
    cc[:, 4 * NCT:5 * NCT] = tilec(sqinv)
    cc[:, 20] = MAGIC

    in_maps = []
    if use_fp32r:
        for c in range(NCORE):
            in_maps.append({
                "xq": np.ascontiguousarray(xp[c * BPC:(c + 1) * BPC]),
                "Wt": Wt,
                "cc": cc,
            })
    else:
        xh = xp.astype(ml_dtypes.bfloat16)
        xl = (xp - xh.astype(np.float32)).astype(ml_dtypes.bfloat16)
        for c in range(NCORE):
            in_maps.append({
                "xh": np.ascontiguousarray(xh[c * BPC:(c + 1) * BPC]),
                "xl": np.ascontiguousarray(xl[c * BPC:(c + 1) * BPC]),
                "Wt": Wt.astype(ml_dtypes.bfloat16),
                "cc": cc,
            })

    res = _build_and_run(in_maps, use_fp32r, eps_eff, trace)
    _last_results["exec_time_ns"] = res.exec_time_ns
    _last_results["mean_exec_time_ns"] = res.mean_exec_time_ns

    out = np.empty((B, COUT, L), dtype=np.float32)
    for c in range(NCORE):
        o = res.results[c]["out"]          # [BPC, NCT, 128, L]
        out[c * BPC:(c + 1) * BPC] = o.reshape(BPC, COUT, L)
    return out


# revision 10
# speedup vs baseline: 1.0581x; 1.0581x over previous
"""BitConvBlock kernel for 8x Trainium2 NeuronCores (SPMD, batch-sharded).

Reference computation (per sample):
  Wq = ternary-quantized W (BitNet b1.58: s = mean|W|, T = clip(round(W/(s+eps)),-1,1), Wq = s*T)
  y  = conv1d(x, Wq, pad=3)                      [B=16, Cout=512, L=8192]
  yn = GroupNorm(1 group, per-channel affine)(y)
  out= yn + sin(alpha*yn + phase)^2 / (alpha+eps)

Strategy:
  - Batch-parallel: 16 samples / 8 cores = 2 samples per core. GroupNorm is
    per-sample, so no collectives.
  - Conv as matmul: y[co,l] = sum_{k,ci} T[co,ci,k] * x[ci, l+k-3], scale s
    folded into the GN epsilon (GN normalization cancels a global scale;
    only eps needs rescaling by 1/s^2).
  - Weights are exactly representable in bf16 ({-1,0,1}); activations are
    split x = hi + lo (bf16 each) and accumulated with 2 matmuls per tile in
    fp32 PSUM -> ~1e-6 relative error at full bf16 PE throughput.
  - Conv output y is spilled to DRAM scratch tiles; after per-sample stats
    (sum / sum-of-squares accumulated on the fly), a second pass applies the
    GN affine + snake activation (ACT sin with magic-number range reduction).
"""
import os
import numpy as np
import ml_dtypes
from contextlib import ExitStack

# ---------------------------------------------------------------- constants
B, CIN, COUT, K, L = 16, 512, 512, 7, 8192
PAD = 3
EPS_Q, EPS_GN, EPS_A = 1e-5, 1e-5, 1e-9
NCORE = 8
BPC = B // NCORE          # samples per core
NCT = COUT // 128         # 4 co tiles
NCI = CIN // 128          # 4 ci tiles
LW = 512                  # conv l-window (one fp32 PSUM bank)
NLW = L // LW             # 16 windows
LP = L + 2 * PAD          # padded length 8198
BW = 1024                 # phase-B tile width
NBW = L // BW             # 8 phase-B tiles per (sample, co_t)
NELEM = COUT * L          # GN reduction size per sample
TWO_PI = 6.283185307179586
INV_2PI = 1.0 / TWO_PI
MAGIC = 12582912.0        # 1.5 * 2**23: float32 round-to-nearest-even trick

MM_DTYPE = os.environ.get("BITCONV_MM", "bf16")   # "bf16" (2-pass) | "fp32r"

_last_results = {}


def _ternary(W: np.ndarray):
    """Bit-exact replica of the reference's _ternary_quant (value part)."""
    s = None
    try:
        import jax
        import jax.numpy as jnp

        cpus = jax.devices("cpu")
        with jax.default_device(cpus[0]):
            s = float(jnp.mean(jnp.abs(jnp.asarray(W))))
    except Exception:
        s = float(np.mean(np.abs(W), dtype=np.float32))
    s32 = np.float32(s)
    q = (W / (s32 + np.float32(EPS_Q))).astype(np.float32)
    T = np.clip(np.rint(q), -1.0, 1.0).astype(np.float32)
    return T, float(s32)


def _build_and_run(in_maps, use_fp32r: bool, eps_eff: float, trace: bool):
    import concourse.bass as bass
    import concourse.tile as tile
    import concourse.mybir as mybir

    # walrus here accepts only one sync-wait per instruction; split waits.
    import bass_rust
    from concourse.vector_clock import ScopedClock, VectorClock

    _orig_commit = tile.TileContext._commit_and_lower
    _skip = (tile.BassTileRelease, tile.BassTileBranchHintPlaceholder,
             tile.BassTileCriticalSection)

    def _commit_split(self, inst, original_block, old_bb_map, bb_to_exit_bb):
        si = getattr(inst, "sync_info", None)
        if (si is not None and len(si.on_wait) > 1
                and not isinstance(inst, _skip)
                and not bass.is_branch_inst(inst)
                and inst.engine != mybir.EngineType.Unassigned):
            waits = list(si.on_wait)
            plain = [w for w in waits
                     if w.sync_type == "semaphore" and w.wait_reg is None]
            rest = [w for w in waits
                    if not (w.sync_type == "semaphore" and w.wait_reg is None)]
            if len(rest) <= 1 and plain:
                keep = rest if rest else [plain.pop()]
                for w in plain:
                    ev = mybir.InstEventSemaphore(
                        name=self.nc.get_next_instruction_name(), ins=[], outs=[])
                    ev.engine = inst.engine
                    ev.sync_info = bass_rust.SyncInfo(on_wait=[w], on_update=[])
                    self._commit_instruction(ev, lazy_reg_writes=False)
                inst.sync_info = bass_rust.SyncInfo(
                    on_wait=keep, on_update=list(si.on_update))
        return _orig_commit(self, inst, original_block, old_bb_map, bb_to_exit_bb)

    def _drain_split(self, tick_clock, wait_clock):
        g = tick_clock.global_clock
        n = len(g)
        for p in range(n):
            t = g[p]
            if t == 0:
                continue
            vec = [0] * n
            vec[p] = t
            d = self.nc.sync.drain()
            wait_clock.add_sem_waits(d.ins, ScopedClock({None: VectorClock(vec)}))
        self.nc.sync.drain()
        self.nc.all_engine_barrier()
        assert self.sems is not None
        popped = self.nc._tile_sem_poison_stack.pop()
        assert popped is self._sem_poison
        self.nc.clear_and_free_semaphores(list(self.sems.allocated().values()))
        self.nc.all_engine_barrier()

    tile.TileContext._commit_and_lower = _commit_split
    tile.TileContext._drain_and_barrier = _drain_split

    from concourse.bass_utils import run_bass_kernel_spmd

    f32 = mybir.dt.float32
    bf16 = mybir.dt.bfloat16
    f32r = mybir.dt.float32r
    AF = mybir.ActivationFunctionType
    ALU = mybir.AluOpType
    AX = mybir.AxisListType

    nc = bass.Bass("TRN2", target_bir_lowering=False, debug=False)

    if use_fp32r:
        x_in = nc.dram_tensor("xq", [BPC, NCI, 128, LP], f32r, kind="ExternalInput").ap()
        w_in = nc.dram_tensor("Wt", [128, K * NCI * NCT * 128], f32r, kind="ExternalInput").ap()
    else:
        xh_in = nc.dram_tensor("xh", [BPC, NCI, 128, LP], bf16, kind="ExternalInput").ap()
        xl_in = nc.dram_tensor("xl", [BPC, NCI, 128, LP], bf16, kind="ExternalInput").ap()
        w_in = nc.dram_tensor("Wt", [128, K * NCI * NCT * 128], bf16, kind="ExternalInput").ap()
    cc_in = nc.dram_tensor("cc", [128, 22], f32, kind="ExternalInput").ap()
    out_ap = nc.dram_tensor("out", [BPC, NCT, 128, L], f32, kind="ExternalOutput").ap()

    wdt = f32r if use_fp32r else bf16

    def widx(k, ci, ct):
        return ((ct * K + k) * NCI + ci) * 128

    with tile.TileContext(nc) as tc:
        with ExitStack() as ctx:
            wpool = ctx.enter_context(tc.tile_pool(name="w", bufs=1))
            cpool = ctx.enter_context(tc.tile_pool(name="consts", bufs=1))
            xpool = ctx.enter_context(tc.tile_pool(name="x", bufs=3))
            cps = ctx.enter_context(tc.tile_pool(name="cps", bufs=6, space="PSUM"))
            sps = ctx.enter_context(tc.tile_pool(name="sps", bufs=2, space="PSUM"))
            ypool = ctx.enter_context(tc.tile_pool(name="ysb", bufs=8))
            qpool = ctx.enter_context(tc.tile_pool(name="sqd", bufs=2))
            stpool = ctx.enter_context(tc.tile_pool(name="st", bufs=2))
            smpool = ctx.enter_context(tc.tile_pool(name="sm", bufs=2))
            bpool = ctx.enter_context(tc.tile_pool(name="bp", bufs=2 if use_fp32r else 3))
            ydram = ctx.enter_context(tc.tile_pool(name="ydram", bufs=2 * NCT * NBW, space="DRAM"))

            W_sb = wpool.tile([128, K * NCI * NCT * 128], wdt)
            nc.sync.dma_start(W_sb[:], w_in[:])
            cc_sb = cpool.tile([128, 22], f32)
            nc.sync.dma_start(cc_sb[:], cc_in[:])
            gnw_c = cc_sb[:, 0:NCT]
            gnb_c = cc_sb[:, NCT:2 * NCT]
            alp_c = cc_sb[:, 2 * NCT:3 * NCT]
            phs_c = cc_sb[:, 3 * NCT:4 * NCT]
            sqi_c = cc_sb[:, 4 * NCT:5 * NCT]
            magic_c = cc_sb[:, 20:21]
            negmagic_c = cc_sb[:, 21:22]
            ones_sb = cpool.tile([128, 128], f32)
            nc.vector.memset(ones_sb[:], 1.0)

            for smp in range(BPC):
                st_sb = stpool.tile([128, 2 * NCT * NLW], f32, tag="st")
                ytiles = {}
                for ct in range(NCT):
                    for g in range(NBW):
                        ytiles[(ct, g)] = ydram.tile([128, BW], f32, name=f"yd_{smp}_{ct}_{g}", tag="yd")

                # ---- phase A: conv + stats ----
                for lw in range(NLW):
                    l0 = lw * LW
                    if use_fp32r:
                        xq_t = []
                        for ci in range(NCI):
                            t = xpool.tile([128, LW + 2 * PAD], f32r, tag=f"xq{ci}")
                            nc.sync.dma_start(t[:], x_in[smp, ci, :, l0:l0 + LW + 2 * PAD])
                            xq_t.append(t)
                    else:
                        xh_t, xl_t = [], []
                        for ci in range(NCI):
                            t = xpool.tile([128, LW + 2 * PAD], bf16, tag=f"xh{ci}")
                            nc.sync.dma_start(t[:], xh_in[smp, ci, :, l0:l0 + LW + 2 * PAD])
                            xh_t.append(t)
                            t = xpool.tile([128, LW + 2 * PAD], bf16, tag=f"xl{ci}")
                            nc.sync.dma_start(t[:], xl_in[smp, ci, :, l0:l0 + LW + 2 * PAD])
                            xl_t.append(t)

                    for ct in range(NCT):
                        ps = cps.tile([128, LW], f32, tag="cpsum")
                        for ci in range(NCI):
                            for k in range(K):
                                w_ap = W_sb[:, widx(k, ci, ct):widx(k, ci, ct) + 128]
                                first = ci == 0 and k == 0
                                last = ci == NCI - 1 and k == K - 1
                                if use_fp32r:
                                    nc.tensor.matmul(ps[:], w_ap, xq_t[ci][:, k:k + LW],
                                                     start=first, stop=last)
                                else:
                                    nc.tensor.matmul(ps[:], w_ap, xh_t[ci][:, k:k + LW],
                                                     start=first, stop=False)
                                    nc.tensor.matmul(ps[:], w_ap, xl_t[ci][:, k:k + LW],
                                                     start=False, stop=last)
                        idx = ct * NLW + lw
                        y_sb = ypool.tile([128, LW], f32, tag="ysb")
                        nc.vector.tensor_scalar(
                            y_sb[:], ps[:], 1.0, 0.0, ALU.mult, ALU.add,
                            accum_out=st_sb[:, idx:idx + 1])
                        sqd = qpool.tile([128, LW], f32, tag="sqd")
                        nc.scalar.activation(
                            sqd[:], ps[:], AF.Square,
                            accum_out=st_sb[:, NCT * NLW + idx:NCT * NLW + idx + 1])
                        g, o = lw // (BW // LW), (lw % (BW // LW)) * LW
                        nc.sync.dma_start(ytiles[(ct, g)][:, o:o + LW], y_sb[:])

                # ---- stats -> per-channel affine ----
                red = smpool.tile([128, 2], f32, tag="red")
                nc.vector.reduce_sum(red[:, 0:1], st_sb[:, 0:NCT * NLW], axis=AX.X)
                nc.vector.reduce_sum(red[:, 1:2], st_sb[:, NCT * NLW:2 * NCT * NLW], axis=AX.X)
                stps = sps.tile([128, 2], f32, tag="stps")
                nc.tensor.matmul(stps[:], ones_sb[:], red[:, 0:2], start=True, stop=True)
                mv = smpool.tile([128, 2], f32, tag="mv")
                nc.vector.tensor_scalar_mul(mv[:], stps[:], 1.0 / NELEM)
                musq = smpool.tile([128, 1], f32, tag="musq")
                nc.vector.tensor_mul(musq[:], mv[:, 0:1], mv[:, 0:1])
                var = smpool.tile([128, 1], f32, tag="var")
                nc.vector.tensor_sub(var[:], mv[:, 1:2], musq[:])
                nc.vector.tensor_scalar_add(var[:], var[:], float(eps_eff))
                std = smpool.tile([128, 1], f32, tag="std")
                nc.scalar.activation(std[:], var[:], AF.Sqrt)
                rv = smpool.tile([128, 1], f32, tag="rv")
                nc.vector.reciprocal(rv[:], std[:])
                Av = smpool.tile([128, NCT], f32, tag="Av")
                nc.vector.tensor_scalar_mul(Av[:], gnw_c, rv[:])
                negmu = smpool.tile([128, 1], f32, tag="negmu")
                nc.vector.tensor_scalar_mul(negmu[:], mv[:, 0:1], -1.0)
                Bv = smpool.tile([128, NCT], f32, tag="Bv")
                nc.vector.tensor_scalar_mul(Bv[:], Av[:], negmu[:])
                nc.vector.tensor_add(Bv[:], Bv[:], gnb_c)

                # ---- phase B: GN affine + snake ----
                for ct in range(NCT):
                    for g in range(NBW):
                        yin = bpool.tile([128, BW], f32, tag="yin")
                        nc.sync.dma_start(yin[:], ytiles[(ct, g)][:])
                        yn = bpool.tile([128, BW], f32, tag="yn")
                        nc.vector.tensor_scalar(
                            yn[:], yin[:], Av[:, ct:ct + 1], Bv[:, ct:ct + 1],
                            ALU.mult, ALU.add)
                        u = bpool.tile([128, BW], f32, tag="u")
                        nc.vector.tensor_scalar(
                            u[:], yn[:], alp_c[:, ct:ct + 1], phs_c[:, ct:ct + 1],
                            ALU.mult, ALU.add)
                        z = bpool.tile([128, BW], f32, tag="z")
                        nc.scalar.activation(z[:], u[:], AF.Identity,
                                             bias=magic_c, scale=INV_2PI)
                        kx = bpool.tile([128, BW], f32, tag="kx")
                        nc.scalar.activation(kx[:], z[:], AF.Identity, bias=negmagic_c)
                        # redt = 2*pi*k - u; sin is odd and gets squared, so the
                        # flipped sign is harmless
                        redt = bpool.tile([128, BW], f32, tag="redt")
                        nc.vector.scalar_tensor_tensor(
                            redt[:], kx[:], TWO_PI, u[:], ALU.mult, ALU.subtract)
                        sg = bpool.tile([128, BW], f32, tag="sg")
                        nc.scalar.activation(sg[:], redt[:], AF.Sin)
                        sq2 = bpool.tile([128, BW], f32, tag="sq2")
                        nc.scalar.activation(sq2[:], sg[:], AF.Square,
                                             scale=sqi_c[:, ct:ct + 1])
                        outt = bpool.tile([128, BW], f32, tag="outt")
                        nc.vector.tensor_add(outt[:], yn[:], sq2[:])
                        nc.sync.dma_start(out_ap[smp, ct, :, g * BW:(g + 1) * BW], outt[:])

    if trace:
        _install_profile_shim()
    res = run_bass_kernel_spmd(nc, in_maps, list(range(NCORE)), trace=trace)
    return res


def _install_profile_shim():
    """Register antenv.axon_hooks so trace=True captures NTFF profiles via the
    axon .so (profiling only; never needed for plain execution)."""
    import sys, types, importlib.util

    if "antenv.axon_hooks" in sys.modules:
        return
    try:
        holder = {"hook": None}
        mod = types.ModuleType("antenv.axon_hooks")
        mod.set_axon_ntff_profile_hook = lambda h: holder.__setitem__("hook", h)
        mod.get_axon_ntff_profile_hook = lambda: holder["hook"]
        import antenv

        spec = importlib.util.spec_from_file_location(
            "trn_boot_shim", "/root/.axon_site/trn_agent_boot/trn_boot.py")
        boot = importlib.util.module_from_spec(spec)
        spec.loader.exec_module(boot)
        hook = boot._ntff_profile_via_ctypes("/opt/axon/libaxon_pjrt.so")
        if hook is None:
            return
        mod.set_axon_ntff_profile_hook(hook)
        sys.modules["antenv.axon_hooks"] = mod
        antenv.axon_hooks = mod
    except Exception:
        pass


def kernel(x, W, gn_w, gn_b, alpha, phase):
    x = np.asarray(x, dtype=np.float32)
    W = np.asarray(W, dtype=np.float32)
    gn_w = np.asarray(gn_w, dtype=np.float32)
    gn_b = np.asarray(gn_b, dtype=np.float32)
    alpha = np.asarray(alpha, dtype=np.float32)
    phase = np.asarray(phase, dtype=np.float32)

    use_fp32r = MM_DTYPE == "fp32r"
    trace = bool(int(os.environ.get("BITCONV_TRACE", "0")))

    T, s = _ternary(W)   # T in {-1,0,1}, conv scale s folded into GN eps
    eps_eff = float(EPS_GN / (np.float64(s) ** 2))

    # weight layout: Wt[ci_in_tile, (k, ci_t, co_t, co)] = T[co, ci, k]
    Tr = T.reshape(NCT, 128, NCI, 128, K)          # [co_t, co, ci_t, ci, k]
    # Wt[ci_in_tile, (co_t, k, ci_t, co)] = T[co, ci, k]   (ct-major for chunked DMA)
    Wt = np.ascontiguousarray(Tr.transpose(3, 0, 4, 2, 1)).reshape(128, -1)

    # padded activations, partition-tiled
    xp = np.zeros((B, CIN, LP), dtype=np.float32)
    xp[:, :, PAD:PAD + L] = x
    xp = xp.reshape(B, NCI, 128, LP)

    # per-channel constants [128, col]
    def tilec(v):
        return np.ascontiguousarray(v.reshape(NCT, 128).T)  # [128, NCT]

    sqinv = np.sqrt(1.0 / (alpha.astype(np.float64) + EPS_A)).astype(np.float32)
    cc = np.zeros((128, 22), dtype=np.float32)
    cc[:, 0:NCT] = tilec(gn_w)
    cc[:, NCT:2 * NCT] = tilec(gn_b)
    cc[:, 2 * NCT:3 * NCT] = tilec(alpha)
    cc[:, 3 * NCT:4 * NCT] = tilec(phase)
    cc[:, 4 * NCT:5 * NCT] = tilec(sqinv)
    cc[:, 20] = MAGIC
    cc[:, 21] = -MAGIC

    in_maps = []
    if use_fp32r:
        for c in range(NCORE):
            in_maps.append({
                "xq": np.ascontiguousarray(xp[c * BPC:(c + 1) * BPC]),
                "Wt": Wt,
                "cc": cc,
            })
    else:
        xh = xp.astype(ml_dtypes.bfloat16)
        xl = (xp - xh.astype(np.float32)).astype(ml_dtypes.bfloat16)
        for c in range(NCORE):
            in_maps.append({
                "xh": np.ascontiguousarray(xh[c * BPC:(c + 1) * BPC]),
                "xl": np.ascontiguousarray(xl[c * BPC:(c + 1) * BPC]),
                "Wt": Wt.astype(ml_dtypes.bfloat16),
                "cc": cc,
            })

    res = _build_and_run(in_maps, use_fp32r, eps_eff, trace)
    _last_results["exec_time_ns"] = res.exec_time_ns
    _last_results["mean_exec_time_ns"] = res.mean_exec_time_ns

    out = np.empty((B, COUT, L), dtype=np.float32)
    for c in range(NCORE):
        o = res.results[c]["out"]          # [BPC, NCT, 128, L]
        out[c * BPC:(c + 1) * BPC] = o.reshape(BPC, COUT, L)
    return out


# revision 11
# speedup vs baseline: 1.1312x; 1.0691x over previous
"""BitConvBlock kernel for 8x Trainium2 NeuronCores (SPMD, batch-sharded).

Reference computation (per sample):
  Wq = ternary-quantized W (BitNet b1.58: s = mean|W|, T = clip(round(W/(s+eps)),-1,1), Wq = s*T)
  y  = conv1d(x, Wq, pad=3)                      [B=16, Cout=512, L=8192]
  yn = GroupNorm(1 group, per-channel affine)(y)
  out= yn + sin(alpha*yn + phase)^2 / (alpha+eps)

Strategy:
  - Batch-parallel: 16 samples / 8 cores = 2 samples per core. GroupNorm is
    per-sample, so no collectives.
  - Conv as matmul: y[co,l] = sum_{k,ci} T[co,ci,k] * x[ci, l+k-3], scale s
    folded into the GN epsilon (GN normalization cancels a global scale;
    only eps needs rescaling by 1/s^2).
  - Weights are exactly representable in bf16 ({-1,0,1}); activations are
    split x = hi + lo (bf16 each) and accumulated with 2 matmuls per tile in
    fp32 PSUM -> ~1e-6 relative error at full bf16 PE throughput.
  - Conv output y is spilled to DRAM scratch tiles; after per-sample stats
    (sum / sum-of-squares accumulated on the fly), a second pass applies the
    GN affine + snake activation (ACT sin with magic-number range reduction).
"""
import os
import numpy as np
import ml_dtypes
from contextlib import ExitStack

# ---------------------------------------------------------------- constants
B, CIN, COUT, K, L = 16, 512, 512, 7, 8192
PAD = 3
EPS_Q, EPS_GN, EPS_A = 1e-5, 1e-5, 1e-9
NCORE = 8
BPC = B // NCORE          # samples per core
NCT = COUT // 128         # 4 co tiles
NCI = CIN // 128          # 4 ci tiles
LW = 512                  # conv l-window (one fp32 PSUM bank)
NLW = L // LW             # 16 windows
LP = L + 2 * PAD          # padded length 8198
BW = 1024                 # phase-B tile width
NBW = L // BW             # 8 phase-B tiles per (sample, co_t)
NELEM = COUT * L          # GN reduction size per sample
TWO_PI = 6.283185307179586
INV_2PI = 1.0 / TWO_PI
MAGIC = 12582912.0        # 1.5 * 2**23: float32 round-to-nearest-even trick

MM_DTYPE = os.environ.get("BITCONV_MM", "bf16")   # "bf16" (2-pass) | "fp32r"

_last_results = {}


def _ternary(W: np.ndarray):
    """Bit-exact replica of the reference's _ternary_quant (value part)."""
    s = None
    try:
        import jax
        import jax.numpy as jnp

        cpus = jax.devices("cpu")
        with jax.default_device(cpus[0]):
            s = float(jnp.mean(jnp.abs(jnp.asarray(W))))
    except Exception:
        s = float(np.mean(np.abs(W), dtype=np.float32))
    s32 = np.float32(s)
    q = (W / (s32 + np.float32(EPS_Q))).astype(np.float32)
    T = np.clip(np.rint(q), -1.0, 1.0).astype(np.float32)
    return T, float(s32)


def _build_and_run(in_maps, use_fp32r: bool, eps_eff: float, trace: bool):
    import concourse.bass as bass
    import concourse.tile as tile
    import concourse.mybir as mybir

    # walrus here accepts only one sync-wait per instruction; split waits.
    import bass_rust
    from concourse.vector_clock import ScopedClock, VectorClock

    _orig_commit = tile.TileContext._commit_and_lower
    _skip = (tile.BassTileRelease, tile.BassTileBranchHintPlaceholder,
             tile.BassTileCriticalSection)

    def _commit_split(self, inst, original_block, old_bb_map, bb_to_exit_bb):
        si = getattr(inst, "sync_info", None)
        if (si is not None and len(si.on_wait) > 1
                and not isinstance(inst, _skip)
                and not bass.is_branch_inst(inst)
                and inst.engine != mybir.EngineType.Unassigned):
            waits = list(si.on_wait)
            plain = [w for w in waits
                     if w.sync_type == "semaphore" and w.wait_reg is None]
            rest = [w for w in waits
                    if not (w.sync_type == "semaphore" and w.wait_reg is None)]
            if len(rest) <= 1 and plain:
                keep = rest if rest else [plain.pop()]
                for w in plain:
                    ev = mybir.InstEventSemaphore(
                        name=self.nc.get_next_instruction_name(), ins=[], outs=[])
                    ev.engine = inst.engine
                    ev.sync_info = bass_rust.SyncInfo(on_wait=[w], on_update=[])
                    self._commit_instruction(ev, lazy_reg_writes=False)
                inst.sync_info = bass_rust.SyncInfo(
                    on_wait=keep, on_update=list(si.on_update))
        return _orig_commit(self, inst, original_block, old_bb_map, bb_to_exit_bb)

    def _drain_split(self, tick_clock, wait_clock):
        g = tick_clock.global_clock
        n = len(g)
        for p in range(n):
            t = g[p]
            if t == 0:
                continue
            vec = [0] * n
            vec[p] = t
            d = self.nc.sync.drain()
            wait_clock.add_sem_waits(d.ins, ScopedClock({None: VectorClock(vec)}))
        self.nc.sync.drain()
        self.nc.all_engine_barrier()
        assert self.sems is not None
        popped = self.nc._tile_sem_poison_stack.pop()
        assert popped is self._sem_poison
        self.nc.clear_and_free_semaphores(list(self.sems.allocated().values()))
        self.nc.all_engine_barrier()

    tile.TileContext._commit_and_lower = _commit_split
    tile.TileContext._drain_and_barrier = _drain_split

    from concourse.bass_utils import run_bass_kernel_spmd

    f32 = mybir.dt.float32
    bf16 = mybir.dt.bfloat16
    f32r = mybir.dt.float32r
    AF = mybir.ActivationFunctionType
    ALU = mybir.AluOpType
    AX = mybir.AxisListType

    nc = bass.Bass("TRN2", target_bir_lowering=False, debug=False)

    if use_fp32r:
        x_in = nc.dram_tensor("xq", [BPC, NCI, 128, LP], f32r, kind="ExternalInput").ap()
        w_in = nc.dram_tensor("Wt", [128, K * NCI * NCT * 128], f32r, kind="ExternalInput").ap()
    else:
        xh_in = nc.dram_tensor("xh", [BPC, NCI, 128, LP], bf16, kind="ExternalInput").ap()
        xl_in = nc.dram_tensor("xl", [BPC, NCI, 128, LP], bf16, kind="ExternalInput").ap()
        w_in = nc.dram_tensor("Wt", [128, K * NCI * NCT * 128], bf16, kind="ExternalInput").ap()
    cc_in = nc.dram_tensor("cc", [128, 22], f32, kind="ExternalInput").ap()
    out_ap = nc.dram_tensor("out", [BPC, NCT, 128, L], f32, kind="ExternalOutput").ap()

    wdt = f32r if use_fp32r else bf16

    def widx(k, ci, ct):
        return ((ct * K + k) * NCI + ci) * 128

    with tile.TileContext(nc) as tc:
        with ExitStack() as ctx:
            wpool = ctx.enter_context(tc.tile_pool(name="w", bufs=1))
            cpool = ctx.enter_context(tc.tile_pool(name="consts", bufs=1))
            xpool = ctx.enter_context(tc.tile_pool(name="x", bufs=3))
            cps = ctx.enter_context(tc.tile_pool(name="cps", bufs=6, space="PSUM"))
            sps = ctx.enter_context(tc.tile_pool(name="sps", bufs=2, space="PSUM"))
            ypool = ctx.enter_context(tc.tile_pool(name="ysb", bufs=8))
            qpool = ctx.enter_context(tc.tile_pool(name="sqd", bufs=2))
            stpool = ctx.enter_context(tc.tile_pool(name="st", bufs=2))
            smpool = ctx.enter_context(tc.tile_pool(name="sm", bufs=2))
            bpool = ctx.enter_context(tc.tile_pool(name="bp", bufs=3))
            ydram = ctx.enter_context(tc.tile_pool(name="ydram", bufs=2 * NCT * NBW, space="DRAM"))

            W_sb = wpool.tile([128, K * NCI * NCT * 128], wdt)
            nc.sync.dma_start(W_sb[:], w_in[:])
            cc_sb = cpool.tile([128, 22], f32)
            nc.sync.dma_start(cc_sb[:], cc_in[:])
            gnw_c = cc_sb[:, 0:NCT]
            gnb_c = cc_sb[:, NCT:2 * NCT]
            alp_c = cc_sb[:, 2 * NCT:3 * NCT]
            phs_c = cc_sb[:, 3 * NCT:4 * NCT]
            sqi_c = cc_sb[:, 4 * NCT:5 * NCT]
            magic_c = cc_sb[:, 20:21]
            negmagic_c = cc_sb[:, 21:22]
            ones_sb = cpool.tile([128, 128], f32)
            nc.vector.memset(ones_sb[:], 1.0)

            for smp in range(BPC):
                st_sb = stpool.tile([128, 2 * NCT * NLW], f32, tag="st")
                ytiles = {}
                for ct in range(NCT):
                    for g in range(NBW):
                        ytiles[(ct, g)] = ydram.tile([128, BW], f32, name=f"yd_{smp}_{ct}_{g}", tag="yd")

                # ---- phase A: conv + stats ----
                for lw in range(NLW):
                    l0 = lw * LW
                    if use_fp32r:
                        xq_t = []
                        for ci in range(NCI):
                            t = xpool.tile([128, LW + 2 * PAD], f32r, tag=f"xq{ci}")
                            nc.sync.dma_start(t[:], x_in[smp, ci, :, l0:l0 + LW + 2 * PAD])
                            xq_t.append(t)
                    else:
                        xh_t, xl_t = [], []
                        for ci in range(NCI):
                            t = xpool.tile([128, LW + 2 * PAD], bf16, tag=f"xh{ci}")
                            nc.sync.dma_start(t[:], xh_in[smp, ci, :, l0:l0 + LW + 2 * PAD])
                            xh_t.append(t)
                            t = xpool.tile([128, LW + 2 * PAD], bf16, tag=f"xl{ci}")
                            nc.sync.dma_start(t[:], xl_in[smp, ci, :, l0:l0 + LW + 2 * PAD])
                            xl_t.append(t)

                    for ct in range(NCT):
                        ps = cps.tile([128, LW], f32, tag="cpsum")
                        for ci in range(NCI):
                            for k in range(K):
                                w_ap = W_sb[:, widx(k, ci, ct):widx(k, ci, ct) + 128]
                                first = ci == 0 and k == 0
                                last = ci == NCI - 1 and k == K - 1
                                if use_fp32r:
                                    nc.tensor.matmul(ps[:], w_ap, xq_t[ci][:, k:k + LW],
                                                     start=first, stop=last)
                                else:
                                    nc.tensor.matmul(ps[:], w_ap, xh_t[ci][:, k:k + LW],
                                                     start=first, stop=False)
                                    nc.tensor.matmul(ps[:], w_ap, xl_t[ci][:, k:k + LW],
                                                     start=False, stop=last)
                        idx = ct * NLW + lw
                        y_sb = ypool.tile([128, LW], f32, tag="ysb")
                        nc.vector.tensor_scalar(
                            y_sb[:], ps[:], 1.0, 0.0, ALU.mult, ALU.add,
                            accum_out=st_sb[:, idx:idx + 1])
                        sqd = qpool.tile([128, LW], f32, tag="sqd")
                        nc.scalar.activation(
                            sqd[:], ps[:], AF.Square,
                            accum_out=st_sb[:, NCT * NLW + idx:NCT * NLW + idx + 1])
                        g, o = lw // (BW // LW), (lw % (BW // LW)) * LW
                        nc.sync.dma_start(ytiles[(ct, g)][:, o:o + LW], y_sb[:])

                # ---- stats -> per-channel affine ----
                red = smpool.tile([128, 2], f32, tag="red")
                nc.vector.reduce_sum(red[:, 0:1], st_sb[:, 0:NCT * NLW], axis=AX.X)
                nc.vector.reduce_sum(red[:, 1:2], st_sb[:, NCT * NLW:2 * NCT * NLW], axis=AX.X)
                stps = sps.tile([128, 2], f32, tag="stps")
                nc.tensor.matmul(stps[:], ones_sb[:], red[:, 0:2], start=True, stop=True)
                mv = smpool.tile([128, 2], f32, tag="mv")
                nc.vector.tensor_scalar_mul(mv[:], stps[:], 1.0 / NELEM)
                musq = smpool.tile([128, 1], f32, tag="musq")
                nc.vector.tensor_mul(musq[:], mv[:, 0:1], mv[:, 0:1])
                var = smpool.tile([128, 1], f32, tag="var")
                nc.vector.tensor_sub(var[:], mv[:, 1:2], musq[:])
                nc.vector.tensor_scalar_add(var[:], var[:], float(eps_eff))
                std = smpool.tile([128, 1], f32, tag="std")
                nc.scalar.activation(std[:], var[:], AF.Sqrt)
                rv = smpool.tile([128, 1], f32, tag="rv")
                nc.vector.reciprocal(rv[:], std[:])
                Av = smpool.tile([128, NCT], f32, tag="Av")
                nc.vector.tensor_scalar_mul(Av[:], gnw_c, rv[:])
                negmu = smpool.tile([128, 1], f32, tag="negmu")
                nc.vector.tensor_scalar_mul(negmu[:], mv[:, 0:1], -1.0)
                Bv = smpool.tile([128, NCT], f32, tag="Bv")
                nc.vector.tensor_scalar_mul(Bv[:], Av[:], negmu[:])
                nc.vector.tensor_add(Bv[:], Bv[:], gnb_c)

                # ---- phase B: GN affine + snake ----
                for ct in range(NCT):
                    for g in range(NBW):
                        yin = bpool.tile([128, BW], f32, tag="yin")
                        nc.sync.dma_start(yin[:], ytiles[(ct, g)][:])
                        yn = bpool.tile([128, BW], f32, tag="yn")
                        nc.vector.tensor_scalar(
                            yn[:], yin[:], Av[:, ct:ct + 1], Bv[:, ct:ct + 1],
                            ALU.mult, ALU.add)
                        u = bpool.tile([128, BW], f32, tag="u")
                        nc.vector.tensor_scalar(
                            u[:], yn[:], alp_c[:, ct:ct + 1], phs_c[:, ct:ct + 1],
                            ALU.mult, ALU.add)
                        z = bpool.tile([128, BW], f32, tag="z")
                        nc.scalar.activation(z[:], u[:], AF.Identity, bias=magic_c)
                        # redt = k - u' (turns); sin arg = 2*pi*redt, the sign
                        # flip is harmless since sin is odd and gets squared
                        redt = bpool.tile([128, BW], f32, tag="redt")
                        nc.vector.scalar_tensor_tensor(
                            redt[:], z[:], MAGIC, u[:], ALU.subtract, ALU.subtract)
                        sg = bpool.tile([128, BW], f32, tag="sg")
                        nc.scalar.activation(sg[:], redt[:], AF.Sin, scale=TWO_PI)
                        sq2 = bpool.tile([128, BW], f32, tag="sq2")
                        nc.scalar.activation(sq2[:], sg[:], AF.Square,
                                             scale=sqi_c[:, ct:ct + 1])
                        outt = bpool.tile([128, BW], f32, tag="outt")
                        nc.vector.tensor_add(outt[:], yn[:], sq2[:])
                        nc.sync.dma_start(out_ap[smp, ct, :, g * BW:(g + 1) * BW], outt[:])

    if trace:
        _install_profile_shim()
    res = run_bass_kernel_spmd(nc, in_maps, list(range(NCORE)), trace=trace)
    return res


def _install_profile_shim():
    """Register antenv.axon_hooks so trace=True captures NTFF profiles via the
    axon .so (profiling only; never needed for plain execution)."""
    import sys, types, importlib.util

    if "antenv.axon_hooks" in sys.modules:
        return
    try:
        holder = {"hook": None}
        mod = types.ModuleType("antenv.axon_hooks")
        mod.set_axon_ntff_profile_hook = lambda h: holder.__setitem__("hook", h)
        mod.get_axon_ntff_profile_hook = lambda: holder["hook"]
        import antenv

        spec = importlib.util.spec_from_file_location(
            "trn_boot_shim", "/root/.axon_site/trn_agent_boot/trn_boot.py")
        boot = importlib.util.module_from_spec(spec)
        spec.loader.exec_module(boot)
        hook = boot._ntff_profile_via_ctypes("/opt/axon/libaxon_pjrt.so")
        if hook is None:
            return
        mod.set_axon_ntff_profile_hook(hook)
        sys.modules["antenv.axon_hooks"] = mod
        antenv.axon_hooks = mod
    except Exception:
        pass


def kernel(x, W, gn_w, gn_b, alpha, phase):
    x = np.asarray(x, dtype=np.float32)
    W = np.asarray(W, dtype=np.float32)
    gn_w = np.asarray(gn_w, dtype=np.float32)
    gn_b = np.asarray(gn_b, dtype=np.float32)
    alpha = np.asarray(alpha, dtype=np.float32)
    phase = np.asarray(phase, dtype=np.float32)

    use_fp32r = MM_DTYPE == "fp32r"
    trace = bool(int(os.environ.get("BITCONV_TRACE", "0")))

    T, s = _ternary(W)   # T in {-1,0,1}, conv scale s folded into GN eps
    eps_eff = float(EPS_GN / (np.float64(s) ** 2))

    # weight layout: Wt[ci_in_tile, (k, ci_t, co_t, co)] = T[co, ci, k]
    Tr = T.reshape(NCT, 128, NCI, 128, K)          # [co_t, co, ci_t, ci, k]
    # Wt[ci_in_tile, (co_t, k, ci_t, co)] = T[co, ci, k]   (ct-major for chunked DMA)
    Wt = np.ascontiguousarray(Tr.transpose(3, 0, 4, 2, 1)).reshape(128, -1)

    # padded activations, partition-tiled
    xp = np.zeros((B, CIN, LP), dtype=np.float32)
    xp[:, :, PAD:PAD + L] = x
    xp = xp.reshape(B, NCI, 128, LP)

    # per-channel constants [128, col]
    def tilec(v):
        return np.ascontiguousarray(v.reshape(NCT, 128).T)  # [128, NCT]

    sqinv = np.sqrt(1.0 / (alpha.astype(np.float64) + EPS_A)).astype(np.float32)
    cc = np.zeros((128, 22), dtype=np.float32)
    cc[:, 0:NCT] = tilec(gn_w)
    cc[:, NCT:2 * NCT] = tilec(gn_b)
    cc[:, 2 * NCT:3 * NCT] = tilec((alpha.astype(np.float64) / (2 * np.pi)).astype(np.float32))
    cc[:, 3 * NCT:4 * NCT] = tilec((phase.astype(np.float64) / (2 * np.pi)).astype(np.float32))
    cc[:, 4 * NCT:5 * NCT] = tilec(sqinv)
    cc[:, 20] = MAGIC
    cc[:, 21] = -MAGIC

    in_maps = []
    if use_fp32r:
        for c in range(NCORE):
            in_maps.append({
                "xq": np.ascontiguousarray(xp[c * BPC:(c + 1) * BPC]),
                "Wt": Wt,
                "cc": cc,
            })
    else:
        xh = xp.astype(ml_dtypes.bfloat16)
        xl = (xp - xh.astype(np.float32)).astype(ml_dtypes.bfloat16)
        for c in range(NCORE):
            in_maps.append({
                "xh": np.ascontiguousarray(xh[c * BPC:(c + 1) * BPC]),
                "xl": np.ascontiguousarray(xl[c * BPC:(c + 1) * BPC]),
                "Wt": Wt.astype(ml_dtypes.bfloat16),
                "cc": cc,
            })

    res = _build_and_run(in_maps, use_fp32r, eps_eff, trace)
    _last_results["exec_time_ns"] = res.exec_time_ns
    _last_results["mean_exec_time_ns"] = res.mean_exec_time_ns

    out = np.empty((B, COUT, L), dtype=np.float32)
    for c in range(NCORE):
        o = res.results[c]["out"]          # [BPC, NCT, 128, L]
        out[c * BPC:(c + 1) * BPC] = o.reshape(BPC, COUT, L)
    return out
